# revision 62
# baseline (speedup 1.0000x reference)
"""CumulativeRadonFeatures Trainium2 kernel.

Computes, for X [32,128,4096], W [100,128], min/max_vals [100]:
    a = einsum('bcl,pc->bpl', X, W)                      # [B,P,L]
    thr[q,p] = min[p] + (max[p]-min[p]) * q/(Q+1), q=1..Q
    cdf[b,p,q] = mean_l(a[b,p,l] < thr[q,p])
    return cdf.reshape(B, P*Q)

Strategy: data-parallel over batch across 8 NeuronCores (4 batches/core).
Per core, per batch:
  - PE matmul with W pre-scaled by s_p = (Q+1)/(max_p-min_p), so PSUM holds
    v = s_p * a. In "u-space" (u = v - s_p*min_p) the Q thresholds are the
    universal integers 1..20.
  - PSUM -> SBUF fp16 copies apply the per-partition bias (free affine on the
    Scalar engine; batch 0 uses DVE, which is otherwise idle at the head),
    producing u. fp16 in u-space keeps per-entry count error ~1e-2 relative
    worst case (fp16 ulp is tiny near the low, rel-err-sensitive thresholds).
  - Counting: one fused compare+accumulate instruction per threshold:
    DVE does 16 thresholds via tensor_scalar(is_lt, accum_out) on fp16 u
    (4x perf mode); ACT counts q=0,1 exactly in fp32 from PSUM and q=2,3
    from fp16 u, via Sign activation with bias + accum_out.
Raw accumulator sums are written out; the host maps them to cdf values.
"""

import numpy as np

B, C, L = 32, 128, 4096
P, Q = 100, 20
N_CORES = 8
B_LOC = B // N_CORES  # 4
L_CHUNK = 512
L_HALF = 2048

# per-batch engine split: ACT counts thresholds [0, n_act), DVE [n_act, Q)
_N_ACT = [4, 4, 4, 4]

_CACHED_NC = None


def _build_program():
    import concourse.bacc as bacc
    import concourse.mybir as mybir
    from concourse.tile import TileContext

    f32 = mybir.dt.float32
    f16 = mybir.dt.float16

    nc = bacc.Bacc(None)

    x = nc.dram_tensor("x", [B_LOC, C, L], f32, kind="ExternalInput")
    wt = nc.dram_tensor("wt", [C, P], f32, kind="ExternalInput")      # (s_p*W_p)^T
    bias = nc.dram_tensor("bias", [P, 1], f32, kind="ExternalInput")  # -s_p*min_p
    # biases for exact fp32 sign passes on PSUM: bias[p]-(q+1) for q=0,1
    abias = nc.dram_tensor("abias", [P, 2], f32, kind="ExternalInput")
    out_d = nc.dram_tensor("out_d", [P, B_LOC * Q], f32, kind="ExternalOutput")
    out_a = nc.dram_tensor("out_a", [P, B_LOC * Q], f32, kind="ExternalOutput")
    # sign sums for q=0,1 per (batch, half): exact fp32 from PSUM
    out_x = nc.dram_tensor("out_x", [P, B_LOC * 4], f32, kind="ExternalOutput")

    with TileContext(nc) as tc:
        with (
            tc.tile_pool(name="singles", bufs=1) as singles,
            tc.tile_pool(name="xin", bufs=12) as xin,
            tc.tile_pool(name="upool", bufs=4) as upool,
            tc.tile_pool(name="gpool", bufs=1) as gpool,
            tc.tile_pool(name="psum", bufs=2, space="PSUM") as psum,
        ):
            # First X chunk's DMA goes out first so the opening matmul isn't
            # queued behind the weight/bias transfers.
            x0_t = xin.tile([C, L_CHUNK], f32, tag="x")
            nc.sync.dma_start(out=x0_t[:], in_=x[0, :, 0:L_CHUNK])
            wt_t = singles.tile([C, P], f32)
            nc.sync.dma_start(out=wt_t[:], in_=wt[:])
            bias_t = singles.tile([P, 1], f32)
            nc.sync.dma_start(out=bias_t[:], in_=bias[:])
            abias_t = singles.tile([P, 2], f32)
            nc.sync.dma_start(out=abias_t[:], in_=abias[:])
            # cnt_d: DVE counts; cnt_a: ACT sign sums (separate tiles so the
            # engines never share a write target)
            cnt_d = singles.tile([P, B_LOC * Q], f32)
            cnt_a = singles.tile([P, B_LOC * Q], f32)
            cnt_x = singles.tile([P, B_LOC * 4], f32)
            nc.gpsimd.memset(cnt_d[:], 0.0)
            nc.gpsimd.memset(cnt_a[:], 0.0)
            nc.gpsimd.memset(cnt_x[:], 0.0)
            # per-threshold ACT biases -(q+1), uniform across partitions
            nq_t = singles.tile([P, Q], f32)
            for q in range(Q):
                nc.gpsimd.memset(nq_t[:, q:q + 1], -float(q + 1))

            g_dve = gpool.tile([P, L], f16, tag="g_dve")
            g_act = gpool.tile([P, L], f16, tag="g_act")

            # Warmup Sign on a tiny tile: pulls the ACT table load to t~0
            # instead of queueing it behind the first batch's X DMAs.
            warm = singles.tile([P, 1], f32)
            nc.scalar.activation(warm[:], nq_t[:, 0:1],
                                 mybir.ActivationFunctionType.Sign)

            first = True
            for b in range(B_LOC):
                u_sb = upool.tile([P, L], f16, tag="u")
                ps_tiles = []
                for h in range(2):
                    ps = psum.tile([P, L_HALF], f32, tag="ps")
                    ps_tiles.append(ps)
                    if first:
                        # Dummy 1-col matmul consumes the wt DMA semaphore on
                        # the PE so real matmuls never carry two DMA waits
                        # (walrus allows one sync wait on the LDWEIGHTS struct).
                        nc.tensor.matmul(ps[:, 0:1], wt_t[:], wt_t[:, 0:1],
                                         start=True, stop=True)
                        first = False
                    for k in range(4):
                        if b == 0 and h == 0 and k == 0:
                            x_t = x0_t
                        else:
                            x_t = xin.tile([C, L_CHUNK], f32, tag="x")
                            nc.sync.dma_start(
                                out=x_t[:],
                                in_=x[b, :, h * L_HALF + k * L_CHUNK:
                                     h * L_HALF + (k + 1) * L_CHUNK],
                            )
                        nc.tensor.matmul(
                            ps[:, k * L_CHUNK:(k + 1) * L_CHUNK],
                            wt_t[:], x_t[:], start=True, stop=True,
                        )
                n_act = _N_ACT[b]
                col = b * Q
                for h in range(2):
                    ps = ps_tiles[h]
                    uh = u_sb[:, h * L_HALF:(h + 1) * L_HALF]
                    # u = v + bias (fp32 PSUM -> fp16 SBUF). Batch 0 on DVE
                    # (idle at head); later batches use ACT's free affine.
                    if b == 0:
                        nc.vector.tensor_scalar(
                            uh, ps[:], bias_t[:], None, mybir.AluOpType.add,
                        )
                    else:
                        nc.scalar.activation(
                            uh, ps[:],
                            mybir.ActivationFunctionType.Identity,
                            bias=bias_t[:], scale=1.0,
                        )
                    # exact fp32 sign passes for the two smallest quantiles,
                    # straight from PSUM (rel-error-sensitive entries)
                    for q in range(2):
                        cx = b * 4 + 2 * q + h
                        nc.scalar.activation(
                            g_act[:, :L_HALF], ps[:],
                            mybir.ActivationFunctionType.Sign,
                            bias=abias_t[:, q:q + 1], scale=1.0,
                            accum_out=cnt_x[:, cx:cx + 1],
                        )

                # ACT thresholds q=2..n_act-1: accum = sum sign(u - (q+1))
                for q in range(2, n_act):
                    nc.scalar.activation(
                        g_act[:], u_sb[:],
                        mybir.ActivationFunctionType.Sign,
                        bias=nq_t[:, q:q + 1], scale=1.0,
                        accum_out=cnt_a[:, col + q:col + q + 1],
                    )
                # DVE thresholds on fp16 u (4x mode): count = sum(u < q+1)
                for q in range(n_act, Q):
                    nc.vector.tensor_scalar(
                        g_dve[:],
                        u_sb[:],
                        float(q + 1),
                        None,
                        mybir.AluOpType.is_lt,
                        mybir.AluOpType.add,
                        accum_out=cnt_d[:, col + q:col + q + 1],
                    )

            nc.sync.dma_start(out=out_d[:], in_=cnt_d[:])
            nc.sync.dma_start(out=out_a[:], in_=cnt_a[:])
            nc.sync.dma_start(out=out_x[:], in_=cnt_x[:])

    if not nc.is_finalized():
        nc.finalize()
    return nc


def _host_scale_bias(min_vals, max_vals):
    """u-space transform: u = s_p * a - s_p * min_p with s_p = (Q+1)/(max-min).

    Reference thresholds: thr_q = min + (max-min) * (q+1)/(Q+1)  (q 0-indexed)
    so a < thr_q  <=>  u < q+1 exactly (s_p > 0)."""
    mn = np.asarray(min_vals, dtype=np.float32)
    mx = np.asarray(max_vals, dtype=np.float32)
    d = mx - mn
    d = np.where(d == 0, np.float32(1.0), d)  # guard degenerate ranges
    s = np.float32(Q + 1) / d
    bias = -s * mn
    return s.astype(np.float32), bias.astype(np.float32)


last_results = None  # BassKernelResults of the most recent run (for profiling)


def kernel(X, W, min_vals, max_vals):
    global _CACHED_NC, last_results
    from concourse.bass_utils import run_bass_kernel_spmd

    X = np.ascontiguousarray(np.asarray(X, dtype=np.float32))
    W = np.asarray(W, dtype=np.float32)

    s, bias = _host_scale_bias(min_vals, max_vals)           # [P], [P]
    wt = np.ascontiguousarray((W * s[:, None]).T)            # [C, P] scaled
    bias_col = np.ascontiguousarray(bias[:, None])           # [P, 1]
    abias = np.ascontiguousarray(
        bias[:, None] - np.arange(1, 3, dtype=np.float32)[None, :])  # [P, 2]

    if _CACHED_NC is None:
        _CACHED_NC = _build_program()
    nc = _CACHED_NC

    in_maps = []
    for i in range(N_CORES):
        in_maps.append({
            "x": X[i * B_LOC:(i + 1) * B_LOC],
            "wt": wt,
            "bias": bias_col,
            "abias": abias,
        })

    res = run_bass_kernel_spmd(nc, in_maps, core_ids=list(range(N_CORES)))
    last_results = res

    cdf = np.empty((B, P, Q), dtype=np.float32)
    inv_l = np.float32(1.0) / np.float32(L)
    for i in range(N_CORES):
        raw_d = res.results[i]["out_d"].reshape(P, B_LOC, Q)
        raw_a = res.results[i]["out_a"].reshape(P, B_LOC, Q)
        raw_x = res.results[i]["out_x"].reshape(P, B_LOC, 2, 2)
        for bl in range(B_LOC):
            b = i * B_LOC + bl
            n_act = _N_ACT[bl]
            # sgn = (L - cnt) - cnt  ->  cnt = (L - sgn) / 2
            for q in range(2):
                sgn = raw_x[:, bl, q, 0] + raw_x[:, bl, q, 1]
                cdf[b, :, q] = (np.float32(L) - sgn) * (inv_l * np.float32(0.5))
            for q in range(2, n_act):
                cdf[b, :, q] = (np.float32(L) - raw_a[:, bl, q]) * \
                    (inv_l * np.float32(0.5))
            for q in range(n_act, Q):
                cdf[b, :, q] = raw_d[:, bl, q] * inv_l
    return cdf.reshape(B, P * Q)
